# revision 12
# baseline (speedup 1.0000x reference)
"""Trainium2 Bass kernel: retrieval-kNN memory system (fp8 PE-scoring design).

Computation (see reference):
  sims = cosine(query, memory_keys[m])  for m in 0..65535
  idx  = top_32(sims); mem_summary = mean(memory_values[idx], axis=0)
  out  = fusion_w @ concat([core_output, study_output, mem_summary]) + fusion_b

Distribution over 8 NeuronCores (row-shard keys/values, row-shard fusion):
  - Keys are pre-transposed + cast to fp8e4 on host: [E, MS] per core. The
    PE streams them as the moving operand against the fp8 query (stationary,
    one column per 128-row e-chunk), accumulating 16 m-slices of [1,512]
    dots in col-tiled PSUM slots (4 banks x 4 col-groups). This reads keys
    at 1 byte/elem -- the kernel is HBM-bound, so fp8 halves-of-halves the
    stream time vs fp32.
  - Coarse ranking uses plain dots (no key norms). Exactness is restored by
    a fp32 rescore of the local coarse top-64: quantize-packed (bf16 score
    | 13-bit row index) values go through a max8/match_replace cascade, the
    decoded indices gather the fp32 key/value rows, and exact cosine scores
    are recomputed for just those rows.
  - AllGather of the 8x64 rescored scores; every core takes the global
    top-32 threshold tau, masks its own candidates, and reduces the masked
    value rows with a mask^T @ V matmul. AllReduce yields 32*mem_summary.
  - The fusion matvec y = W @ [co; so; ms/32] + b runs on the PE as 96
    accumulating [1,512] matmuls against host-pretransposed bf16 weight
    chunks (contraction on partitions); the co/so chunks interleave with the
    key stream so only the mem third sits on the tail.
"""

import sys

import numpy as np

try:
    import concourse.bass as _probe  # noqa: F401
except Exception:  # pragma: no cover
    sys.path.insert(0, "/opt/trn_rl_repo")

E = 4096
M = 65536
NCORES = 8
MS = M // NCORES  # 8192 key/value rows per core
ET = E // 128  # 32 e-chunks (PE contraction tiles)
NSL = MS // 512  # 16 m-slices of 512 dots
KC = 64  # coarse candidates rescored per core
TOPK = 32
WROWS = E // NCORES  # 512 fusion output rows per core
CCH = E // 128  # 32 contraction chunks per fusion third

_CACHED_NC = None


def build_module():
    import concourse.bacc as bacc
    import concourse.bass as bass
    import concourse.mybir as mybir
    import concourse.tile as tile

    f32 = mybir.dt.float32
    f8 = mybir.dt.float8e4
    bf = mybir.dt.bfloat16
    i32 = mybir.dt.int32
    Alu = mybir.AluOpType
    Act = mybir.ActivationFunctionType
    groups = [list(range(NCORES))]

    nc = bacc.Bacc(
        "TRN2", target_bir_lowering=False, debug=False, num_devices=NCORES
    )

    kT8 = nc.declare_dram_parameter("kT8", [E, MS], f8, isOutput=False)
    k32 = nc.declare_dram_parameter("k32", [MS, E], f32, isOutput=False)
    v32 = nc.declare_dram_parameter("v32", [MS, E], f32, isOutput=False)
    w1t = nc.declare_dram_parameter("w1t", [128, CCH * 512], bf, isOutput=False)
    w2t = nc.declare_dram_parameter("w2t", [128, CCH * 512], bf, isOutput=False)
    w3t = nc.declare_dram_parameter("w3t", [128, CCH * 512], bf, isOutput=False)
    q8 = nc.declare_dram_parameter("q8", [128, CCH], f8, isOutput=False)
    co16 = nc.declare_dram_parameter("co16", [128, CCH], bf, isOutput=False)
    so16 = nc.declare_dram_parameter("so16", [128, CCH], bf, isOutput=False)
    qb64 = nc.declare_dram_parameter("qb64", [KC, E], f32, isOutput=False)
    biasp = nc.declare_dram_parameter("biasp", [1, WROWS], f32, isOutput=False)
    ident = nc.declare_dram_parameter("ident", [128, 128], f32, isOutput=False)
    iotas = nc.declare_dram_parameter("iotas", [128, 64], f32, isOutput=False)
    outp = nc.declare_dram_parameter("out", [WROWS], f32, isOutput=True)

    with tile.TileContext(nc) as tc:
        with (
            tc.tile_pool(name="keys", bufs=3) as kp,
            tc.tile_pool(name="persist", bufs=1) as sp,
            tc.tile_pool(name="psum_s", bufs=1, space="PSUM") as pps,
            tc.tile_pool(name="psum_t", bufs=1, space="PSUM") as pp,
            tc.tile_pool(name="psum_mp", bufs=1, space="PSUM") as pm,
            tc.tile_pool(name="dram", bufs=1, space="DRAM") as dp,
        ):
            # ---- persistent SBUF ----
            w1s = sp.tile([128, CCH * 512], bf, tag="w1s")
            w2s = sp.tile([128, CCH * 512], bf, tag="w2s")
            w3s = sp.tile([128, CCH * 512], bf, tag="w3s")
            q8s = sp.tile([128, CCH], f8, tag="q8s")
            cos = sp.tile([128, CCH], bf, tag="cos")
            sos = sp.tile([128, CCH], bf, tag="sos")
            qbs = sp.tile([KC, E], f32, tag="qbs")
            ids = sp.tile([128, 128], f32, tag="ids")
            iot = sp.tile([128, 64], f32, tag="iot")
            bia = sp.tile([1, WROWS], f32, tag="bia")
            ones_col = sp.tile([1, KC], f32, tag="ones")
            scs = [
                sp.tile([128, 512], f32, tag=f"scs{b}", name=f"scs{b}")
                for b in range(4)
            ]
            scores = sp.tile([128, 64], f32, tag="scores")
            qv = sp.tile([128, 64], bf, tag="qv")
            qvf = sp.tile([128, 64], f32, tag="qvf")
            vv = sp.tile([128, 64], f32, tag="vv")
            c1 = sp.tile([128, 8], f32, tag="c1")
            c1t = sp.tile([8, 128], f32, tag="c1t")
            c2 = sp.tile([8, 32], f32, tag="c2")
            allv = sp.tile([1, 256], f32, tag="allv")
            w64 = sp.tile([1, KC], f32, tag="w64")
            wq16 = sp.tile([1, KC], bf, tag="wq16")
            wq32 = sp.tile([1, KC], f32, tag="wq32")
            idxf = sp.tile([1, KC], f32, tag="idxf")
            idxp = sp.tile([KC, 1], i32, tag="idxp")
            gK = sp.tile([KC, E], f32, tag="gK")
            gV = sp.tile([KC, E], f32, tag="gV")
            dump64 = sp.tile([KC, 1], f32, tag="dump64")
            rsd0 = sp.tile([KC, 1], f32, tag="rsd0")
            rsd1 = sp.tile([KC, 1], f32, tag="rsd1")
            rsn0 = sp.tile([KC, 1], f32, tag="rsn0")
            rsn1 = sp.tile([KC, 1], f32, tag="rsn1")
            rsn = sp.tile([KC, 1], f32, tag="rsn")
            rs = sp.tile([KC, 1], f32, tag="rs")
            rs_row = sp.tile([1, KC], f32, tag="rs_row")
            agv = sp.tile([1, NCORES * KC], f32, tag="agv")
            w32 = sp.tile([1, 32], f32, tag="w32")
            tau64 = sp.tile([KC, 1], f32, tag="tau64")
            mask = sp.tile([KC, 1], f32, tag="mask")
            partial2 = [
                sp.tile([128, 512], f32, tag=f"par{b}", name=f"par{b}")
                for b in range(2)
            ]
            ms32 = sp.tile([32, 128], f32, tag="ms32")
            msb = sp.tile([128, 32], bf, tag="msb")
            y_sb = sp.tile([1, WROWS], f32, tag="y_sb")

            # ---- persistent PSUM: 4 score banks + fusion accumulator ----
            scp = [
                pps.tile([128, 512], f32, tag=f"scp{b}", name=f"scp{b}")
                for b in range(4)
            ]
            y_ps = pps.tile([128, 512], f32, tag="y_ps")

            # ---- DRAM bounce buffers ----
            sc_d = dp.tile([MS], f32, tag="sc_d")
            c1_d = dp.tile([128 * 8], f32, tag="c1_d")
            c2_d = dp.tile([8 * 32], f32, tag="c2_d")
            ag_in = dp.tile([KC], f32, tag="ag_in")
            ag_out = dp.tile([NCORES * KC], f32, tag="ag_out")
            ar_in = dp.tile([E], f32, tag="ar_in")
            ar_out = dp.tile([E], f32, tag="ar_out")

            # ---- small loads ----
            nc.sync.dma_start(out=q8s[:], in_=q8[:])
            nc.sync.dma_start(out=cos[:], in_=co16[:])
            nc.sync.dma_start(out=sos[:], in_=so16[:])
            nc.sync.dma_start(out=qbs[:], in_=qb64[:])
            nc.sync.dma_start(out=ids[:], in_=ident[:])
            nc.sync.dma_start(out=iot[:], in_=iotas[:])
            nc.sync.dma_start(out=bia[:], in_=biasp[:])
            nc.vector.memset(ones_col[:], 1.0)
            # fusion weight streams, 1 MB sub-loads for fine-grained deps
            for ws, wp_ in ((w1s, w1t), (w2s, w2t), (w3s, w3t)):
                for u in range(4):
                    sl = slice(4096 * u, 4096 * (u + 1))
                    nc.sync.dma_start(out=ws[:, sl], in_=wp_[:, sl])

            def ymm(lhs_col, rhs_sl, start, stop):
                nc.tensor.matmul(
                    out=y_ps[0:1, :],
                    lhsT=lhs_col,
                    rhs=rhs_sl,
                    start=start,
                    stop=stop,
                    skip_group_check=True,
                )

            # ---- key stream: fp8 PE matvec, 16 col-tiled psum slots ----
            for t in range(ET):
                kt = kp.tile([128, MS], f8, tag="kt")
                nc.sync.dma_start(out=kt[:], in_=kT8[128 * t : 128 * (t + 1), :])
                for s in range(NSL):
                    b, j = s >> 2, s & 3
                    nc.tensor.matmul(
                        out=scp[b][32 * j : 32 * j + 1, :],
                        lhsT=q8s[:, t : t + 1],
                        rhs=kt[:, 512 * s : 512 * (s + 1)],
                        start=(t == 0),
                        stop=(t == ET - 1),
                        tile_position=(0, 32 * j),
                        skip_group_check=True,
                    )
                # interleave fusion co/so matvec chunks once weights are in
                if 8 <= t < 24:
                    c = 2 * (t - 8)
                    ymm(cos[:, c : c + 1], w1s[:, 512 * c : 512 * (c + 1)],
                        start=(c == 0), stop=False)
                    ymm(cos[:, c + 1 : c + 2], w1s[:, 512 * (c + 1) : 512 * (c + 2)],
                        start=False, stop=False)
                if 16 <= t < 32:
                    c = 2 * (t - 16)
                    ymm(sos[:, c : c + 1], w2s[:, 512 * c : 512 * (c + 1)],
                        start=False, stop=False)
                    ymm(sos[:, c + 1 : c + 2], w2s[:, 512 * (c + 1) : 512 * (c + 2)],
                        start=False, stop=False)

            # ---- drain score psum -> [128, 64] via DRAM bounce ----
            sc_v = sc_d[:].rearrange("(r i) -> r i", r=16)
            for b in range(4):
                nc.vector.tensor_copy(out=scs[b][:], in_=scp[b][:])
                # rows {0,32,64,96} -> sc_d rows 4b..4b+3 (DMA partition stride)
                nc.scalar.dma_start(out=sc_v[4 * b : 4 * b + 4], in_=scs[b][0:128:32, :])
            nc.sync.dma_start(out=scores[:], in_=sc_d[:].rearrange("(p t) -> p t", p=128))

            # ---- pack (quantized score | row index) and local top-64 ----
            # v = bf16(dots * 2^-9 + 3) + (p*64 + t) * 2^-21  (exact in fp32)
            nc.vector.tensor_scalar(
                out=qv[:], in0=scores[:], scalar1=2.0**-9, scalar2=3.0,
                op0=Alu.mult, op1=Alu.add,
            )
            nc.vector.tensor_copy(out=qvf[:], in_=qv[:])
            nc.vector.tensor_add(out=vv[:], in0=qvf[:], in1=iot[:])
            nc.vector.max(out=c1[:], in_=vv[:])  # per-partition top-8
            c1_dv = c1_d[:].rearrange("(p j) -> p j", p=128)
            nc.scalar.dma_start(out=c1_dv, in_=c1[:])
            nc.scalar.dma_start(out=c1t[:], in_=c1_d[:].rearrange("(j f) -> j f", j=8))
            for r in range(4):  # top-32 within each 16-partition group
                sl = c2[:, 8 * r : 8 * r + 8]
                nc.vector.max(out=sl, in_=c1t[:])
                nc.vector.match_replace(
                    out=c1t[:], in_to_replace=sl, in_values=c1t[:], imm_value=0.0
                )
            c2_dv = c2_d[:].rearrange("(p j) -> p j", p=8)
            nc.scalar.dma_start(out=c2_dv, in_=c2[:])
            nc.scalar.dma_start(out=allv[:], in_=c2_d[:].rearrange("(j f) -> j f", j=1))
            for r in range(8):  # global top-64 of the 256 survivors
                sl = w64[:, 8 * r : 8 * r + 8]
                nc.vector.max(out=sl, in_=allv[:])
                nc.vector.match_replace(
                    out=allv[:], in_to_replace=sl, in_values=allv[:], imm_value=0.0
                )
            # decode indices: m = (v - bf16(v)) * 2^21
            nc.vector.tensor_copy(out=wq16[:], in_=w64[:])
            nc.vector.tensor_copy(out=wq32[:], in_=wq16[:])
            nc.vector.tensor_sub(out=idxf[:], in0=w64[:], in1=wq32[:])
            nc.vector.tensor_scalar_mul(idxf[:], idxf[:], float(2**21))
            ips = pp.tile([KC, 1], f32, tag="tpa")
            nc.tensor.transpose(out=ips[:], in_=idxf[:], identity=ids[0:1, 0:1])
            nc.vector.tensor_copy(out=idxp[:], in_=ips[:])

            # ---- gather fp32 rows of the 64 candidates; exact rescore ----
            nc.gpsimd.indirect_dma_start(
                out=gK[:],
                out_offset=None,
                in_=k32[:],
                in_offset=bass.IndirectOffsetOnAxis(ap=idxp[:, :1], axis=0),
                bounds_check=MS - 1,
                oob_is_err=False,
            )
            nc.gpsimd.indirect_dma_start(
                out=gV[:],
                out_offset=None,
                in_=v32[:],
                in_offset=bass.IndirectOffsetOnAxis(ap=idxp[:, :1], axis=0),
                bounds_check=MS - 1,
                oob_is_err=False,
            )
            # dots on DVE, sum-of-squares in-place on ACT, split in halves so
            # ACT(H0) overlaps DVE(H1)
            H = E // 2
            nc.vector.scalar_tensor_tensor(
                out=dump64[:].broadcast_to([KC, H]),
                in0=gK[:, 0:H], scalar=1.0, in1=qbs[:, 0:H],
                op0=Alu.mult, op1=Alu.mult, accum_out=rsd0[:],
            )
            nc.vector.scalar_tensor_tensor(
                out=dump64[:].broadcast_to([KC, H]),
                in0=gK[:, H:E], scalar=1.0, in1=qbs[:, H:E],
                op0=Alu.mult, op1=Alu.mult, accum_out=rsd1[:],
            )
            nc.scalar.activation(
                out=gK[:, 0:H], in_=gK[:, 0:H], func=Act.Square, accum_out=rsn0[:]
            )
            nc.scalar.activation(
                out=gK[:, H:E], in_=gK[:, H:E], func=Act.Square, accum_out=rsn1[:]
            )
            nc.vector.tensor_add(out=rsn[:], in0=rsn0[:], in1=rsn1[:])
            nc.scalar.activation(out=rsn[:], in_=rsn[:], func=Act.Sqrt)
            nc.vector.reciprocal(out=rsn[:], in_=rsn[:])
            nc.vector.tensor_add(out=rs[:], in0=rsd0[:], in1=rsd1[:])
            nc.vector.tensor_mul(out=rs[:], in0=rs[:], in1=rsn[:])

            # ---- AllGather rescored scores; global top-32 threshold tau ----
            rps = pp.tile([1, KC], f32, tag="tpb")
            nc.tensor.transpose(out=rps[:], in_=rs[:], identity=ids[0:KC, 0:KC])
            nc.vector.tensor_copy(out=rs_row[:], in_=rps[:])
            nc.scalar.dma_start(out=ag_in[None, :], in_=rs_row[:])
            nc.gpsimd.collective_compute(
                "AllGather",
                Alu.bypass,
                replica_groups=groups,
                ins=[ag_in.opt()],
                outs=[ag_out.opt()],
            )
            nc.sync.dma_start(out=agv[:], in_=ag_out[None, :])
            for r in range(4):
                sl = w32[:, 8 * r : 8 * r + 8]
                nc.vector.max(out=sl, in_=agv[:])
                nc.vector.match_replace(
                    out=agv[:], in_to_replace=sl, in_values=agv[:], imm_value=-1e30
                )
            tps = pp.tile([KC, 1], f32, tag="tpa")
            nc.tensor.matmul(
                out=tps[:], lhsT=ones_col[:], rhs=w32[0:1, 31:32], start=True, stop=True
            )
            nc.vector.tensor_copy(out=tau64[:], in_=tps[:])

            # ---- masked value sum -> AllReduce -> mem third of fusion ----
            nc.vector.tensor_scalar(
                out=mask[:], in0=rs[:], scalar1=tau64[:, :1], scalar2=None,
                op0=Alu.is_ge,
            )
            # 8 chunks land in 2 psum banks x 4 col-groups, drained like scores
            ar_v = ar_in[:].rearrange("(r i) -> r i", r=8)
            for b in range(2):
                mp = pm.tile([128, 512], f32, tag="mp")
                for j in range(4):
                    ch = 4 * b + j
                    nc.tensor.matmul(
                        out=mp[32 * j : 32 * j + 1, :],
                        lhsT=mask[:, 0:1],
                        rhs=gV[:, 512 * ch : 512 * (ch + 1)],
                        start=True, stop=True,
                        tile_position=(0, 32 * j),
                        skip_group_check=True,
                    )
                nc.vector.tensor_copy(out=partial2[b][:], in_=mp[:])
                nc.scalar.dma_start(
                    out=ar_v[4 * b : 4 * b + 4], in_=partial2[b][0:128:32, :]
                )
            nc.gpsimd.collective_compute(
                "AllReduce",
                Alu.add,
                replica_groups=groups,
                ins=[ar_in.opt()],
                outs=[ar_out.opt()],
            )
            nc.sync.dma_start(out=ms32[:], in_=ar_out[:].rearrange("(c p) -> c p", p=128))
            msps = pp.tile([128, 32], f32, tag="tpa")
            nc.tensor.transpose(out=msps[:], in_=ms32[:], identity=ids[0:32, 0:32])
            nc.scalar.activation(
                out=msb[:], in_=msps[:], func=Act.Copy, scale=1.0 / TOPK
            )
            for c in range(CCH):
                ymm(msb[:, c : c + 1], w3s[:, 512 * c : 512 * (c + 1)],
                    start=False, stop=(c == CCH - 1))

            # ---- y = psum + bias -> out ----
            nc.scalar.activation(out=y_sb[:], in_=y_ps[0:1, :], func=Act.Copy)
            nc.vector.tensor_add(out=y_sb[:], in0=y_sb[:], in1=bia[:])
            nc.sync.dma_start(out=outp[None, :], in_=y_sb[:])

    nc.compile()
    return nc


def get_module():
    global _CACHED_NC
    if _CACHED_NC is None:
        _CACHED_NC = build_module()
    return _CACHED_NC


def make_in_maps(
    core_output, study_output, query, memory_keys, memory_values, fusion_w, fusion_b
):
    import ml_dtypes

    f = np.float32
    f8 = ml_dtypes.float8_e4m3
    bf = ml_dtypes.bfloat16

    mk = np.asarray(memory_keys, dtype=f)
    mv = np.asarray(memory_values, dtype=f)
    q = np.asarray(query, dtype=f)
    co = np.asarray(core_output, dtype=f)
    so = np.asarray(study_output, dtype=f)
    fw = np.asarray(fusion_w, dtype=f)
    fb = np.asarray(fusion_b, dtype=f)

    keys8 = mk.astype(f8)  # [M, E]
    # chunked vectors: [p, c] = vec[128c + p]
    q8 = np.ascontiguousarray(q.reshape(CCH, 128).T).astype(f8)
    co16 = np.ascontiguousarray(co.reshape(CCH, 128).T).astype(bf)
    so16 = np.ascontiguousarray(so.reshape(CCH, 128).T).astype(bf)
    qb64 = np.ascontiguousarray(np.broadcast_to(q, (KC, E)))
    identm = np.eye(128, dtype=f)
    iota = (
        64.0 * np.arange(128, dtype=f)[:, None] + np.arange(64, dtype=f)[None, :]
    ) * f(2.0**-21)

    def wpack(w):  # [WROWS, E] slice of fusion_w -> [128, CCH*512] bf16
        wt = np.ascontiguousarray(w.T)  # [E, WROWS]
        return np.ascontiguousarray(
            wt.reshape(CCH, 128, WROWS).transpose(1, 0, 2).reshape(128, CCH * WROWS)
        ).astype(bf)

    in_maps = []
    for c in range(NCORES):
        rows = slice(c * MS, (c + 1) * MS)
        wr = slice(c * WROWS, (c + 1) * WROWS)
        in_maps.append(
            {
                "kT8": np.ascontiguousarray(keys8[rows].T),
                "k32": mk[rows],
                "v32": mv[rows],
                "w1t": wpack(fw[wr, 0:E]),
                "w2t": wpack(fw[wr, E : 2 * E]),
                "w3t": wpack(fw[wr, 2 * E : 3 * E]),
                "q8": q8,
                "co16": co16,
                "so16": so16,
                "qb64": qb64,
                "biasp": fb[wr].reshape(1, WROWS),
                "ident": identm,
                "iotas": iota,
            }
        )
    return in_maps


def kernel(
    core_output,
    study_output,
    query,
    memory_keys,
    memory_values,
    fusion_w,
    fusion_b,
    top_k=TOPK,
    **_unused,
):
    assert int(top_k) == TOPK, f"kernel hardcodes top_k={TOPK}, got {top_k}"
    from concourse.bass_utils import run_bass_kernel_spmd

    nc = get_module()
    in_maps = make_in_maps(
        core_output, study_output, query, memory_keys, memory_values, fusion_w, fusion_b
    )
    res = run_bass_kernel_spmd(nc, in_maps, list(range(NCORES)))
    return np.concatenate([res.results[c]["out"] for c in range(NCORES)], axis=0)
